# revision 23
# baseline (speedup 1.0000x reference)
"""Trainium2 Bass kernel for nn_MlpMixer_18966575579742.

Complex-valued per-frequency (j) MLP:
  o1r = gelu(xr@w1[0] - xi@w1[1] + b1[0]);  o1i = gelu(xi@w1[0] + xr@w1[1] + b1[1])
  o2r = o1r@w2[0] - o1i@w2[1] + b2[0];      o2i = o1i@w2[0] + o1i@w2[1] + b2[1]
  (note: o2i intentionally uses o1i with BOTH w2[0] and w2[1], as in the source)

Sharding over 8 cores: 2 j-halves (13 each) x 4 batch-quarters (B=32 -> 512 rows).

Per-core dataflow, all matmuls in fp16 (1 PE pass/row vs fp32's 4 -- fp16's
10-bit mantissa keeps rel err ~1e-3, far under the 2e-2 gate):
  - host pre-transposes x shards to [j, k, rows] fp16 and pre-builds the six
    fp16 stationary weight tensors actually used on chip:
      w1e = [w1[0], w1[1], -w1[1]]           (layout [j, k, hc, 3, h'])
      w2e = [w2[0], -w2[1], w2[0]+w2[1]]     (layout [j, p, 3, hc, k'])
    plus partition-major fp32 biases b1t [128, 2, j, hc], b2t [128, 2, j]
  - L1: plain 4-matmul complex product accumulated in PSUM (no DVE combines):
      rp = xr@w1[0] + xi@(-w1[1]);  ip = xr@w1[1] + xi@w1[0]
    ScalarE applies exact-erf GELU + per-partition b1 bias straight out of
    PSUM, writing o1 fp16 (partitions = h, kept transposed [h_chunk, rows])
  - L2 (w2 stationary, o1 moving): o2T [k'=128, rows] PSUM via w2[0], -w2[1]
    (real) and w2[0]+w2[1] (imag) -- 3 matmuls per h_chunk, run one j BEHIND
    layer 1 so the PE never stalls waiting on the GELUs of the current j
  - DVE drains PSUM with fused per-partition b2 bias, writing fp16
  - output stays transposed [j, c, k', rows] fp16; host does the final
    transpose + complex interleave
  - DMA queues: x + weights on sync, outputs on gpsimd (last j on sync so
    the end-of-kernel queue drain overlaps); ScalarE runs GELU only (single
    ACT table, no reloads); a fused j0 "boot" DMA (w1-hc0 + xr + xi) plus
    zero-matmul PE warmup hides the DVFS p-state ramp inside the DMA wait
"""

import sys

if "/opt/trn_rl_repo" not in sys.path:
    sys.path.insert(0, "/opt/trn_rl_repo")

import numpy as np

B, I, J, K, F = 128, 16, 26, 128, 4
H = K * F  # 512
NJG = 2  # j groups
NRG = 4  # row (batch) groups
JL = J // NJG  # 13 j per core
BL = B // NRG  # 32 batches per core
ROWS = BL * I  # 512 rows per core
NHC = H // 128  # 4 h-chunks

_cache = {}


def _build_nc():
    from contextlib import ExitStack

    import concourse.mybir as mybir
    import concourse.tile as tile
    from concourse import bacc

    f32 = mybir.dt.float32
    f16 = mybir.dt.float16
    nc = bacc.Bacc(None)

    # x pre-transposed on host: [j, k, rows] fp16
    xr = nc.declare_dram_parameter("xr", [JL, K, ROWS], f16, isOutput=False)
    xi = nc.declare_dram_parameter("xi", [JL, K, ROWS], f16, isOutput=False)
    # stationary weights, fp16, partition(k)-major, packed per h-chunk:
    # [w1[0], w1[1], -w1[1]] slices
    w1e = nc.declare_dram_parameter("w1e", [JL, K, NHC, 3, 128], f16, isOutput=False)
    w2e = nc.declare_dram_parameter("w2e", [JL, 128, 3, NHC, K], f16, isOutput=False)
    # j0 boot block: [w1e[0,:,0], xr[0], xi[0]] fused into one transfer
    boot = nc.declare_dram_parameter("boot", [K, 11, 128], f16, isOutput=False)
    # biases already partition-major fp32
    b1 = nc.declare_dram_parameter("b1", [128, 2, JL, NHC], f32, isOutput=False)
    b2 = nc.declare_dram_parameter("b2", [128, 2, JL], f32, isOutput=False)
    # transposed output: [j, c, k', rows] fp16; host fixes layout
    out = nc.declare_dram_parameter("out", [JL, 2, K, ROWS], f16, isOutput=True)

    GELU = mybir.ActivationFunctionType.Gelu

    with tile.TileContext(nc) as tc, ExitStack() as ctx:
        const = ctx.enter_context(tc.tile_pool(name="const", bufs=1))
        w1p = ctx.enter_context(tc.tile_pool(name="w1p", bufs=3))
        w2p = ctx.enter_context(tc.tile_pool(name="w2p", bufs=3))
        xtp = ctx.enter_context(tc.tile_pool(name="xtp", bufs=3))
        o1p = ctx.enter_context(tc.tile_pool(name="o1p", bufs=2))
        outp = ctx.enter_context(tc.tile_pool(name="outp", bufs=4))
        ps1 = ctx.enter_context(tc.tile_pool(name="ps1", bufs=4, space="PSUM"))
        ps2 = ctx.enter_context(tc.tile_pool(name="ps2", bufs=4, space="PSUM"))

        # warm tile first (cheap DVE memset -- the DVE queue is empty right
        # after the preamble): the PE warmup below is gated only on it
        warm = const.tile([128, 128], f16)
        nc.vector.memset(warm, 0.0)
        # bias tiles are allocated here but DMA'd after j=0's weight blocks:
        # they are needed later (first GELU / first L2 drain) and must not
        # delay the hc1/hc2 weight transfers on the gpsimd queue
        b1t = const.tile([128, 2, JL, NHC], f32)
        b2t = const.tile([128, 2, JL], f32)

        def load_tiles(j):
            w1t = w1p.tile([128, NHC, 3, 128], f16, tag="w1t")  # [k, hc, t, h']
            w2t = w2p.tile([128, 3, NHC, K], f16, tag="w2t")  # [p, t, hc, k']
            nc.sync.dma_start(out=w1t, in_=w1e[j])
            if j == 0:
                # j0's x lives in the boot tile; skip the regular loads
                nc.sync.dma_start(out=w2t, in_=w2e[j])
                return None, None, w1t, w2t
            xtr = xtp.tile([128, ROWS], f16, tag="xtr")
            nc.sync.dma_start(out=xtr, in_=xr[j])
            xti = xtp.tile([128, ROWS], f16, tag="xti")
            nc.sync.dma_start(out=xti, in_=xi[j])
            nc.sync.dma_start(out=w2t, in_=w2e[j])
            return xtr, xti, w1t, w2t

        def warmup(n):
            # dependency-free matmuls on the zeroed warm tile: they execute
            # while the first DMAs are in flight, walking the PE through
            # its DVFS p-state ramp so real matmuls start at full clock
            for _ in range(n):
                scratch = ps1.tile([128, 128], f32, tag="ps1")
                nc.tensor.matmul(scratch, warm, warm, start=True, stop=True)

        def layer1(j, xtr, xti, w1t, bt=None):
            o1r = o1p.tile([128, NHC, ROWS], f16, tag="o1r")
            o1i = o1p.tile([128, NHC, ROWS], f16, tag="o1i")
            for hc in range(NHC):
                # j0/hc0 weights come from the boot tile (arrives first)
                w = bt[:, 0:3] if bt is not None and hc == 0 else w1t[:, hc]
                rp = ps1.tile([128, ROWS], f32, tag="ps1")
                ip = ps1.tile([128, ROWS], f32, tag="ps1")
                nc.tensor.matmul(rp, w[:, 0], xtr, start=True, stop=False)
                nc.tensor.matmul(rp, w[:, 2], xti, start=False, stop=True)
                nc.tensor.matmul(ip, w[:, 1], xtr, start=True, stop=False)
                nc.tensor.matmul(ip, w[:, 0], xti, start=False, stop=True)
                nc.scalar.activation(
                    o1r[:, hc], rp, GELU, bias=b1t[:, 0, j, hc : hc + 1]
                )
                nc.scalar.activation(
                    o1i[:, hc], ip, GELU, bias=b1t[:, 1, j, hc : hc + 1]
                )
            return o1r, o1i

        def layer2(j, o1r, o1i, w2t):
            outq = nc.sync if j == JL - 1 else nc.gpsimd
            p2r = ps2.tile([128, ROWS], f32, tag="ps2")
            p2i = ps2.tile([128, ROWS], f32, tag="ps2")
            for hc in range(NHC):
                first, last = hc == 0, hc == NHC - 1
                # p2i stops first so its drain can overlap p2r's last matmuls
                nc.tensor.matmul(
                    p2i, w2t[:, 2, hc], o1i[:, hc], start=first, stop=last
                )
                nc.tensor.matmul(
                    p2r, w2t[:, 0, hc], o1r[:, hc], start=first, stop=False
                )
                nc.tensor.matmul(
                    p2r, w2t[:, 1, hc], o1i[:, hc], start=False, stop=last
                )
            oti = outp.tile([128, ROWS], f16, tag="ot")
            nc.vector.tensor_scalar_add(oti, p2i, b2t[:, 1, j : j + 1])
            outq.dma_start(out=out[j, 1], in_=oti)
            otr = outp.tile([128, ROWS], f16, tag="ot")
            nc.vector.tensor_scalar_add(otr, p2r, b2t[:, 0, j : j + 1])
            outq.dma_start(out=out[j, 0], in_=otr)

        # j0 boot: one fused DMA carries everything the first h-chunk's
        # matmuls need (w1-hc0 + xr + xi), minimizing time-to-first-matmul
        bt = const.tile([128, 11, 128], f16)
        nc.sync.dma_start(out=bt, in_=boot[:])
        nc.gpsimd.dma_start(out=b1t, in_=b1[:])
        nc.gpsimd.dma_start(out=b2t, in_=b2[:])

        # software pipeline: L2 runs one j behind L1 so the PE never waits
        # on the GELUs of the j it just produced
        prev = None
        for j in range(JL):
            xtr, xti, w1t, w2t = load_tiles(j)
            if j == 0:
                xtr, xti = bt[:, 3:7], bt[:, 7:11]
                warmup(32)
            o1r, o1i = layer1(j, xtr, xti, w1t, bt if j == 0 else None)
            if prev is not None:
                layer2(*prev)
            prev = (j, o1r, o1i, w2t)
        layer2(*prev)

    if not nc.is_finalized():
        nc.finalize()
    return nc


def _prep_weights(w1, b1, w2, b2, js):
    w1c = w1[:, js].reshape(2, JL, K, NHC, 128)  # fp32
    w1e = np.empty((JL, K, NHC, 3, 128), np.float16)
    for hc in range(NHC):
        w1e[:, :, hc, 0] = w1c[0, :, :, hc]
        w1e[:, :, hc, 1] = w1c[1, :, :, hc]
        w1e[:, :, hc, 2] = -w1c[1, :, :, hc]
    w2c = w2[:, js].reshape(2, JL, NHC, 128, K)
    w2e = np.empty((JL, 128, 3, NHC, K), np.float16)
    w2e[:, :, 0] = w2c[0].transpose(0, 2, 1, 3)  # [JL, 128, NHC, K]
    w2e[:, :, 1] = -w2c[1].transpose(0, 2, 1, 3)
    w2e[:, :, 2] = (w2c[0] + w2c[1]).transpose(0, 2, 1, 3)
    b1t = np.ascontiguousarray(
        b1[:, js].reshape(2, JL, NHC, 128).transpose(3, 0, 1, 2), np.float32
    )
    b2t = np.ascontiguousarray(b2[:, js].transpose(2, 0, 1), np.float32)
    return w1e, w2e, b1t, b2t


def _shard_inputs(x_real, x_imag, w1, b1, w2, b2):
    in_maps = []
    for jg in range(NJG):
        js = slice(jg * JL, (jg + 1) * JL)
        w1e, w2e, b1t, b2t = _prep_weights(w1, b1, w2, b2, js)
        for rg in range(NRG):
            bs = slice(rg * BL, (rg + 1) * BL)
            # [BL, I, JL, K] -> [JL, K, BL*I] fp16
            xr_s = np.ascontiguousarray(
                x_real[bs, :, js, :].transpose(2, 3, 0, 1).reshape(JL, K, ROWS),
                np.float16,
            )
            xi_s = np.ascontiguousarray(
                x_imag[bs, :, js, :].transpose(2, 3, 0, 1).reshape(JL, K, ROWS),
                np.float16,
            )
            bootarr = np.empty((K, 11, 128), np.float16)
            bootarr[:, 0:3] = w1e[0, :, 0]
            bootarr[:, 3:7] = xr_s[0].reshape(K, 4, 128)
            bootarr[:, 7:11] = xi_s[0].reshape(K, 4, 128)
            in_maps.append(
                {
                    "xr": xr_s,
                    "xi": xi_s,
                    "boot": bootarr,
                    "w1e": w1e,
                    "w2e": w2e,
                    "b1": b1t,
                    "b2": b2t,
                }
            )
    return in_maps


def _gather(results):
    out = np.empty((B, I, J, K), np.complex64)
    idx = 0
    for jg in range(NJG):
        for rg in range(NRG):
            js = slice(jg * JL, (jg + 1) * JL)
            bs = slice(rg * BL, (rg + 1) * BL)
            o = np.asarray(results[idx]["out"])  # [13, 2, 128, 512] fp16
            oc = o[:, 0].astype(np.complex64)
            oc.imag = o[:, 1].astype(np.float32)
            # [j, k, rows] -> [rows, j, k] -> [BL, I, JL, K]
            out[bs, :, js, :] = oc.transpose(2, 0, 1).reshape(BL, I, JL, K)
            idx += 1
    return out


def run(trace=False, **inputs):
    from concourse.bass_utils import run_bass_kernel_spmd

    if "nc" not in _cache:
        _cache["nc"] = _build_nc()
    in_maps = _shard_inputs(
        np.asarray(inputs["x_real"], np.float32),
        np.asarray(inputs["x_imag"], np.float32),
        np.asarray(inputs["w1"], np.float32),
        np.asarray(inputs["b1"], np.float32),
        np.asarray(inputs["w2"], np.float32),
        np.asarray(inputs["b2"], np.float32),
    )
    res = run_bass_kernel_spmd(_cache["nc"], in_maps, list(range(8)), trace=trace)
    return _gather(res.results), res


def kernel(**inputs):
    out, _ = run(trace=False, **inputs)
    return out


# revision 24
# speedup vs baseline: 1.0215x; 1.0215x over previous
"""Trainium2 Bass kernel for nn_MlpMixer_18966575579742.

Complex-valued per-frequency (j) MLP:
  o1r = gelu(xr@w1[0] - xi@w1[1] + b1[0]);  o1i = gelu(xi@w1[0] + xr@w1[1] + b1[1])
  o2r = o1r@w2[0] - o1i@w2[1] + b2[0];      o2i = o1i@w2[0] + o1i@w2[1] + b2[1]
  (note: o2i intentionally uses o1i with BOTH w2[0] and w2[1], as in the source)

Sharding over 8 cores: 2 j-halves (13 each) x 4 batch-quarters (B=32 -> 512 rows).

Per-core dataflow, all matmuls in fp16 (1 PE pass/row vs fp32's 4 -- fp16's
10-bit mantissa keeps rel err ~1e-3, far under the 2e-2 gate):
  - host pre-transposes x shards to [j, k, rows] fp16 and pre-builds the six
    fp16 stationary weight tensors actually used on chip:
      w1e = [w1[0], w1[1], -w1[1]]           (layout [j, k, hc, 3, h'])
      w2e = [w2[0], -w2[1], w2[0]+w2[1]]     (layout [j, p, 3, hc, k'])
    plus partition-major fp32 biases b1t [128, 2, j, hc], b2t [128, 2, j]
  - L1: plain 4-matmul complex product accumulated in PSUM (no DVE combines):
      rp = xr@w1[0] + xi@(-w1[1]);  ip = xr@w1[1] + xi@w1[0]
    ScalarE applies exact-erf GELU + per-partition b1 bias straight out of
    PSUM, writing o1 fp16 (partitions = h, kept transposed [h_chunk, rows])
  - L2 (w2 stationary, o1 moving): o2T [k'=128, rows] PSUM via w2[0], -w2[1]
    (real) and w2[0]+w2[1] (imag) -- 3 matmuls per h_chunk, run one j BEHIND
    layer 1 so the PE never stalls waiting on the GELUs of the current j
  - DVE drains PSUM with fused per-partition b2 bias, writing fp16
  - output stays transposed [j, c, k', rows] fp16; host does the final
    transpose + complex interleave
  - DMA queues: x + weights on sync, outputs on gpsimd (last j on sync so
    the end-of-kernel queue drain overlaps); ScalarE runs GELU only (single
    ACT table, no reloads); a fused j0 "boot" DMA (w1-hc0 + xr + xi) plus
    zero-matmul PE warmup hides the DVFS p-state ramp inside the DMA wait
"""

import sys

if "/opt/trn_rl_repo" not in sys.path:
    sys.path.insert(0, "/opt/trn_rl_repo")

import numpy as np

B, I, J, K, F = 128, 16, 26, 128, 4
H = K * F  # 512
NJG = 2  # j groups
NRG = 4  # row (batch) groups
JL = J // NJG  # 13 j per core
BL = B // NRG  # 32 batches per core
ROWS = BL * I  # 512 rows per core
NHC = H // 128  # 4 h-chunks

_cache = {}


def _build_nc():
    from contextlib import ExitStack

    import concourse.mybir as mybir
    import concourse.tile as tile
    from concourse import bacc

    f32 = mybir.dt.float32
    f16 = mybir.dt.float16
    nc = bacc.Bacc(None)

    # x pre-transposed on host: [j, k, rows] fp16
    xr = nc.declare_dram_parameter("xr", [JL, K, ROWS], f16, isOutput=False)
    xi = nc.declare_dram_parameter("xi", [JL, K, ROWS], f16, isOutput=False)
    # stationary weights, fp16, partition(k)-major, packed per h-chunk:
    # [w1[0], w1[1], -w1[1]] slices
    w1e = nc.declare_dram_parameter("w1e", [JL, K, NHC, 3, 128], f16, isOutput=False)
    w2e = nc.declare_dram_parameter("w2e", [JL, 128, 3, NHC, K], f16, isOutput=False)
    # j0 boot block: [w1e[0,:,0:2], xr[0], xi[0]] fused into one transfer
    boot = nc.declare_dram_parameter("boot", [K, 14, 128], f16, isOutput=False)
    # biases already partition-major fp32
    b1 = nc.declare_dram_parameter("b1", [128, 2, JL, NHC], f32, isOutput=False)
    b2 = nc.declare_dram_parameter("b2", [128, 2, JL], f32, isOutput=False)
    # transposed output: [j, c, k', rows] fp16; host fixes layout
    out = nc.declare_dram_parameter("out", [JL, 2, K, ROWS], f16, isOutput=True)

    GELU = mybir.ActivationFunctionType.Gelu

    with tile.TileContext(nc) as tc, ExitStack() as ctx:
        const = ctx.enter_context(tc.tile_pool(name="const", bufs=1))
        w1p = ctx.enter_context(tc.tile_pool(name="w1p", bufs=3))
        w2p = ctx.enter_context(tc.tile_pool(name="w2p", bufs=3))
        xtp = ctx.enter_context(tc.tile_pool(name="xtp", bufs=3))
        o1p = ctx.enter_context(tc.tile_pool(name="o1p", bufs=2))
        outp = ctx.enter_context(tc.tile_pool(name="outp", bufs=4))
        ps1 = ctx.enter_context(tc.tile_pool(name="ps1", bufs=4, space="PSUM"))
        ps2 = ctx.enter_context(tc.tile_pool(name="ps2", bufs=4, space="PSUM"))

        # warm tile first (cheap DVE memset -- the DVE queue is empty right
        # after the preamble): the PE warmup below is gated only on it
        warm = const.tile([128, 128], f16)
        nc.vector.memset(warm, 0.0)
        # bias tiles are allocated here but DMA'd after j=0's weight blocks:
        # they are needed later (first GELU / first L2 drain) and must not
        # delay the hc1/hc2 weight transfers on the gpsimd queue
        b1t = const.tile([128, 2, JL, NHC], f32)
        b2t = const.tile([128, 2, JL], f32)

        def load_tiles(j):
            w1t = w1p.tile([128, NHC, 3, 128], f16, tag="w1t")  # [k, hc, t, h']
            w2t = w2p.tile([128, 3, NHC, K], f16, tag="w2t")  # [p, t, hc, k']
            if j == 0:
                # j0's x and hc0/hc1 weights live in the boot tile; only
                # hc2/hc3 come separately (they ride behind boot on sync
                # and still arrive well before the PE reaches them)
                nc.sync.dma_start(out=w1t[:, 2:], in_=w1e[j, :, 2:])
                nc.sync.dma_start(out=w2t, in_=w2e[j])
                return None, None, w1t, w2t
            nc.sync.dma_start(out=w1t, in_=w1e[j])
            xtr = xtp.tile([128, ROWS], f16, tag="xtr")
            nc.sync.dma_start(out=xtr, in_=xr[j])
            xti = xtp.tile([128, ROWS], f16, tag="xti")
            nc.sync.dma_start(out=xti, in_=xi[j])
            nc.sync.dma_start(out=w2t, in_=w2e[j])
            return xtr, xti, w1t, w2t

        def warmup(n):
            # dependency-free matmuls on the zeroed warm tile: they execute
            # while the first DMAs are in flight, walking the PE through
            # its DVFS p-state ramp so real matmuls start at full clock
            for _ in range(n):
                scratch = ps1.tile([128, 128], f32, tag="ps1")
                nc.tensor.matmul(scratch, warm, warm, start=True, stop=True)

        def layer1(j, xtr, xti, w1t, bt=None):
            o1r = o1p.tile([128, NHC, ROWS], f16, tag="o1r")
            o1i = o1p.tile([128, NHC, ROWS], f16, tag="o1i")
            for hc in range(NHC):
                # j0's hc0/hc1 weights come from the boot tile
                w = bt[:, 3 * hc : 3 * hc + 3] if bt is not None and hc < 2 else w1t[:, hc]
                rp = ps1.tile([128, ROWS], f32, tag="ps1")
                ip = ps1.tile([128, ROWS], f32, tag="ps1")
                nc.tensor.matmul(rp, w[:, 0], xtr, start=True, stop=False)
                nc.tensor.matmul(rp, w[:, 2], xti, start=False, stop=True)
                nc.tensor.matmul(ip, w[:, 1], xtr, start=True, stop=False)
                nc.tensor.matmul(ip, w[:, 0], xti, start=False, stop=True)
                nc.scalar.activation(
                    o1r[:, hc], rp, GELU, bias=b1t[:, 0, j, hc : hc + 1]
                )
                nc.scalar.activation(
                    o1i[:, hc], ip, GELU, bias=b1t[:, 1, j, hc : hc + 1]
                )
            return o1r, o1i

        def layer2(j, o1r, o1i, w2t):
            outq = nc.sync if j == JL - 1 else nc.gpsimd
            p2r = ps2.tile([128, ROWS], f32, tag="ps2")
            p2i = ps2.tile([128, ROWS], f32, tag="ps2")
            for hc in range(NHC):
                first, last = hc == 0, hc == NHC - 1
                # p2i stops first so its drain can overlap p2r's last matmuls
                nc.tensor.matmul(
                    p2i, w2t[:, 2, hc], o1i[:, hc], start=first, stop=last
                )
                nc.tensor.matmul(
                    p2r, w2t[:, 0, hc], o1r[:, hc], start=first, stop=False
                )
                nc.tensor.matmul(
                    p2r, w2t[:, 1, hc], o1i[:, hc], start=False, stop=last
                )
            oti = outp.tile([128, ROWS], f16, tag="ot")
            nc.vector.tensor_scalar_add(oti, p2i, b2t[:, 1, j : j + 1])
            outq.dma_start(out=out[j, 1], in_=oti)
            otr = outp.tile([128, ROWS], f16, tag="ot")
            nc.vector.tensor_scalar_add(otr, p2r, b2t[:, 0, j : j + 1])
            outq.dma_start(out=out[j, 0], in_=otr)

        # j0 boot: one fused DMA carries everything the first h-chunk's
        # matmuls need (w1-hc0 + xr + xi), minimizing time-to-first-matmul
        bt = const.tile([128, 14, 128], f16)
        nc.sync.dma_start(out=bt, in_=boot[:])
        nc.gpsimd.dma_start(out=b1t, in_=b1[:])
        nc.gpsimd.dma_start(out=b2t, in_=b2[:])

        # software pipeline: L2 runs one j behind L1 so the PE never waits
        # on the GELUs of the j it just produced
        prev = None
        for j in range(JL):
            xtr, xti, w1t, w2t = load_tiles(j)
            if j == 0:
                xtr, xti = bt[:, 6:10], bt[:, 10:14]
                warmup(29)
            o1r, o1i = layer1(j, xtr, xti, w1t, bt if j == 0 else None)
            if prev is not None:
                layer2(*prev)
            prev = (j, o1r, o1i, w2t)
        layer2(*prev)

    if not nc.is_finalized():
        nc.finalize()
    return nc


def _prep_weights(w1, b1, w2, b2, js):
    w1c = w1[:, js].reshape(2, JL, K, NHC, 128)  # fp32
    w1e = np.empty((JL, K, NHC, 3, 128), np.float16)
    for hc in range(NHC):
        w1e[:, :, hc, 0] = w1c[0, :, :, hc]
        w1e[:, :, hc, 1] = w1c[1, :, :, hc]
        w1e[:, :, hc, 2] = -w1c[1, :, :, hc]
    w2c = w2[:, js].reshape(2, JL, NHC, 128, K)
    w2e = np.empty((JL, 128, 3, NHC, K), np.float16)
    w2e[:, :, 0] = w2c[0].transpose(0, 2, 1, 3)  # [JL, 128, NHC, K]
    w2e[:, :, 1] = -w2c[1].transpose(0, 2, 1, 3)
    w2e[:, :, 2] = (w2c[0] + w2c[1]).transpose(0, 2, 1, 3)
    b1t = np.ascontiguousarray(
        b1[:, js].reshape(2, JL, NHC, 128).transpose(3, 0, 1, 2), np.float32
    )
    b2t = np.ascontiguousarray(b2[:, js].transpose(2, 0, 1), np.float32)
    return w1e, w2e, b1t, b2t


def _shard_inputs(x_real, x_imag, w1, b1, w2, b2):
    in_maps = []
    for jg in range(NJG):
        js = slice(jg * JL, (jg + 1) * JL)
        w1e, w2e, b1t, b2t = _prep_weights(w1, b1, w2, b2, js)
        for rg in range(NRG):
            bs = slice(rg * BL, (rg + 1) * BL)
            # [BL, I, JL, K] -> [JL, K, BL*I] fp16
            xr_s = np.ascontiguousarray(
                x_real[bs, :, js, :].transpose(2, 3, 0, 1).reshape(JL, K, ROWS),
                np.float16,
            )
            xi_s = np.ascontiguousarray(
                x_imag[bs, :, js, :].transpose(2, 3, 0, 1).reshape(JL, K, ROWS),
                np.float16,
            )
            bootarr = np.empty((K, 14, 128), np.float16)
            bootarr[:, 0:3] = w1e[0, :, 0]
            bootarr[:, 3:6] = w1e[0, :, 1]
            bootarr[:, 6:10] = xr_s[0].reshape(K, 4, 128)
            bootarr[:, 10:14] = xi_s[0].reshape(K, 4, 128)
            in_maps.append(
                {
                    "xr": xr_s,
                    "xi": xi_s,
                    "boot": bootarr,
                    "w1e": w1e,
                    "w2e": w2e,
                    "b1": b1t,
                    "b2": b2t,
                }
            )
    return in_maps


def _gather(results):
    out = np.empty((B, I, J, K), np.complex64)
    idx = 0
    for jg in range(NJG):
        for rg in range(NRG):
            js = slice(jg * JL, (jg + 1) * JL)
            bs = slice(rg * BL, (rg + 1) * BL)
            o = np.asarray(results[idx]["out"])  # [13, 2, 128, 512] fp16
            oc = o[:, 0].astype(np.complex64)
            oc.imag = o[:, 1].astype(np.float32)
            # [j, k, rows] -> [rows, j, k] -> [BL, I, JL, K]
            out[bs, :, js, :] = oc.transpose(2, 0, 1).reshape(BL, I, JL, K)
            idx += 1
    return out


def run(trace=False, **inputs):
    from concourse.bass_utils import run_bass_kernel_spmd

    if "nc" not in _cache:
        _cache["nc"] = _build_nc()
    in_maps = _shard_inputs(
        np.asarray(inputs["x_real"], np.float32),
        np.asarray(inputs["x_imag"], np.float32),
        np.asarray(inputs["w1"], np.float32),
        np.asarray(inputs["b1"], np.float32),
        np.asarray(inputs["w2"], np.float32),
        np.asarray(inputs["b2"], np.float32),
    )
    res = run_bass_kernel_spmd(_cache["nc"], in_maps, list(range(8)), trace=trace)
    return _gather(res.results), res


def kernel(**inputs):
    out, _ = run(trace=False, **inputs)
    return out
